# revision 13
# baseline (speedup 1.0000x reference)
"""Trainium2 Bass kernel for nn_Decoder_43336220016932.

Luong-attention LSTM decoder with teacher forcing:
  out[b,t,:] = log_softmax(tanh([ctx_t, h_t] @ W_fc + b_fc))

Strategy (8 NeuronCores):
  - Vocab-sharded tensor parallel: core i owns W_fc[:, i*4000:(i+1)*4000]
    resident in SBUF (fp8). The recurrence (LSTM) is replicated.
  - All recurrent state is kept transposed ([U -> partitions, B -> free])
    so the per-step critical path has no transposes inside the loop.
  - The x-projection (Zx) is computed in 4 t-blocks: block 0 up front,
    blocks 1-3 interleaved into the recurrence so the PE starts the
    serial LSTM chain early.
  - The big FC runs in fp8 (DoubleRow perf mode, 2 k-chunks per
    instruction): W_fc scaled by 256, ctx/h activations scaled by 16,
    un-scaled for free via the tanh activation's scale=1/4096.
  - The h-only half of the FC is interleaved into the recurrence
    (computable as h_t lands), staged to DRAM in bf16 (scaled).
  - D2 computes FC + tanh + exp-sum per 128-row chunk with logits kept
    in SBUF; per-row sumexp stats are AllReduced in 2 batches; batch-0
    finalize is interleaved into late D2 chunks so the collective
    latency hides under compute. Finalize (logits - logZ) alternates
    Vector/Scalar and streams bf16 outputs to DRAM via the gpsimd
    queue (keeps the sync DMA queue free).
  - logits are tanh-bounded in [-1,1], so sumexp uses the fixed shift
    exp(x - 1): no max pass, a single AllReduce(add) per batch.
"""
from contextlib import ExitStack

import numpy as np
import ml_dtypes

import concourse.bass as bass
import concourse.tile as tile
from concourse import bacc, mybir
from concourse.bass_utils import run_bass_kernel_spmd
from concourse.masks import make_identity

B, S, L, U, E, V = 32, 64, 64, 512, 256, 32000
T = S - 1                  # 63 decode steps
NC = 8                     # cores
VS = V // NC               # 4000 vocab shard per core
TB = T * B                 # 2016 (t, b) rows, t-major
G4U = 4 * U                # 2048
BL = B * L                 # 2048
SW = 256.0                 # fp8 scale on W_fc
SA = 16.0                  # fp8 scale on ctx / h
SP = SW * SA               # product scale on logits in PSUM
SWH = 16.0                 # fp8 scale on W_h
AF = mybir.ActivationFunctionType
ALU = mybir.AluOpType
AX = mybir.AxisListType
PM = mybir.MatmulPerfMode
F32 = mybir.dt.float32
BF16 = mybir.dt.bfloat16
FP8 = mybir.dt.float8e4
I32 = mybir.dt.int32

_CACHE = {}


def build(n_cores=NC):
    """Build the SPMD Bass program (same program on every core)."""
    nc = bacc.Bacc("TRN2", target_bir_lowering=False, debug=False,
                   num_devices=n_cores)

    # ---- external I/O ----
    tidx = nc.dram_tensor("tidx", [TB, 1], I32, kind="ExternalInput").ap()
    emb_bf = nc.dram_tensor("emb_bf", [V, E], BF16, kind="ExternalInput").ap()
    wx_bf = nc.dram_tensor("wx_bf", [E + 1, G4U], BF16, kind="ExternalInput").ap()
    wh_q = nc.dram_tensor("wh_q", [U, G4U], FP8, kind="ExternalInput").ap()
    enc_bf = nc.dram_tensor("enc_bf", [B, L, U], BF16, kind="ExternalInput").ap()
    enct_bf = nc.dram_tensor("enct_bf", [U, BL], BF16, kind="ExternalInput").ap()
    wa_bf = nc.dram_tensor("wa_bf", [U, U], BF16, kind="ExternalInput").ap()
    h0 = nc.dram_tensor("h0", [B, U], F32, kind="ExternalInput").ap()
    c0 = nc.dram_tensor("c0", [B, U], F32, kind="ExternalInput").ap()
    wfc_q = nc.dram_tensor("wfc_q", [2 * U, VS], FP8, kind="ExternalInput").ap()
    bfc_bf = nc.dram_tensor("bfc_bf", [1, VS], BF16, kind="ExternalInput").ap()
    out = nc.dram_tensor("out", [B, T, VS], BF16, kind="ExternalOutput").ap()

    with tile.TileContext(nc) as tc, ExitStack() as perm:
        # ---------------- permanent pools ----------------
        konst = perm.enter_context(tc.tile_pool(name="konst", bufs=1))
        wpool = perm.enter_context(tc.tile_pool(name="wpool", bufs=1))
        hpool = perm.enter_context(tc.tile_pool(name="hpool", bufs=1))
        dram = perm.enter_context(tc.tile_pool(name="dram", bufs=1, space="DRAM"))
        stats = perm.enter_context(tc.tile_pool(name="stats", bufs=1))

        idt = konst.tile([128, 128], BF16)
        make_identity(nc, idt[:])
        negone = konst.tile([128, 1], F32)
        nc.vector.memset(negone[:], -1.0)
        idtf = konst.tile([128, 128], F32)
        make_identity(nc, idtf[:])
        ones_bf = konst.tile([1, 512], BF16)
        nc.vector.memset(ones_bf[:], 1.0)

        # resident: W_fc shard (fp8, x256) + bias broadcast (x4096)
        wfc_sb = wpool.tile([128, 8 * VS], FP8)        # [k, k*VS + v]
        wfc_k = wfc_sb[:].rearrange("p (k v) -> p k v", k=8)
        bfc_bc = wpool.tile([128, VS], BF16)

        # H: h.T history (bf16 for gates/scores); H8: fp8 x16 copy for FC.
        # col = k*2048 + slot*32 + b  (k = u chunk of 128)
        H = hpool.tile([128, 4 * 64 * B], BF16)
        Hk = H[:].rearrange("p (k s b) -> p k s b", k=4, s=64)
        H8 = hpool.tile([128, 4 * 64 * B], FP8)
        H8k = H8[:].rearrange("p (k s b) -> p k s b", k=4, s=64)
        H8r = H8[:].rearrange("p (k c) -> p k c", k=4)
        # G_ctx: ctx.T fp8 x16, col = k*2016 + t*32 + b
        Gc = hpool.tile([128, 4 * TB], FP8)
        Gck = Gc[:].rearrange("p (k t b) -> p k t b", k=4, t=T)
        Gcr = Gc[:].rearrange("p (k r) -> p k r", k=4)
        cT = hpool.tile([128, 128], F32)     # c.T state, col = k*32+b

        # per-row ((t,b) grouped [128 x 16]) log-softmax stats.
        lsum_sb = stats.tile([128, 16], F32)   # local sum exp(x - 1)
        sg_sb = stats.tile([128, 16], F32)     # global sum
        logz_sb = stats.tile([128, 16], F32)   # ln(global sum)
        nlz_sb = stats.tile([128, 16], F32)    # -(1 + ln(global sum))
        nc.vector.memset(lsum_sb[:], 1.0)

        # DRAM scratch
        zxt_d = dram.tile([T, 128, 16, B], F32)      # z_x.T per step
        hfc_d = dram.tile([16, 128, VS], BF16)       # h-part FC partials (xSP)
        ccs_in = [dram.tile([128, 8], F32, name=f"cci{i}") for i in range(2)]
        ccs_out = [dram.tile([128, 8], F32, name=f"cco{i}") for i in range(2)]

        with ExitStack() as mid:
            mwp = mid.enter_context(tc.tile_pool(name="midw", bufs=1))
            wh_sb = mwp.tile([128, 4 * G4U], FP8)       # [k, k*2048 + m] xSWH
            wh8r = wh_sb[:].rearrange("p (k m) -> p k m", k=4)
            for k in range(4):
                nc.sync.dma_start(wh_sb[:, k * G4U:(k + 1) * G4U],
                                  wh_q[k * 128:(k + 1) * 128, :])
            epT_sb = mwp.tile([128, 4 * BL], BF16)      # ep.T [u-chunk, (b,l)]
            xt_sb = mwp.tile([128, 2 * TB], BF16)       # x.T [E-chunk, (t,b)]
            wx_sb = mwp.tile([128, 2 * G4U], BF16)
            wxb_sb = mwp.tile([1, G4U], BF16)

            with ExitStack() as pscope:
                psb = pscope.enter_context(tc.tile_pool(name="p_sbuf", bufs=2))
                pps = pscope.enter_context(
                    tc.tile_pool(name="p_psum", bufs=1, space="PSUM"))

                def emit_zx_unit(nb, mi):
                    """One (t-block, m-chunk) unit of Zx.T = W_x.T@X.T + b."""
                    t0 = nb * 16
                    tn = min(16, T - t0)
                    ncols = tn * B
                    zps = pps.tile([128, 512], F32, tag="zx")
                    for k in range(2):
                        nc.tensor.matmul(
                            zps[:, :ncols],
                            wx_sb[:, k * G4U + mi * 128:
                                  k * G4U + (mi + 1) * 128],
                            xt_sb[:, k * TB + t0 * B:
                                  k * TB + t0 * B + ncols],
                            start=(k == 0), stop=False)
                    nc.tensor.matmul(zps[:, :ncols],
                                     wxb_sb[:, mi * 128:(mi + 1) * 128],
                                     ones_bf[:, :ncols],
                                     start=False, stop=True)
                    zsb = psb.tile([128, 512], F32, tag="zxs")
                    nc.vector.tensor_copy(zsb[:, :ncols], zps[:, :ncols])
                    nc.sync.dma_start(
                        zxt_d[t0:t0 + tn, :, mi, :]
                        .rearrange("t p b -> p t b"),
                        zsb[:, :ncols].rearrange("p (t b) -> p t b", b=B))

                def emit_gather(i, tps):
                    r0 = i * 128
                    rows = min(128, TB - r0)
                    ix = psb.tile([128, 1], I32, tag="ix")
                    nc.sync.dma_start(ix[:rows, :], tidx[r0:r0 + rows, :])
                    xg = psb.tile([128, E], BF16, tag="xg")
                    nc.gpsimd.indirect_dma_start(
                        out=xg[:rows, :], out_offset=None,
                        in_=emb_bf[:],
                        in_offset=bass.IndirectOffsetOnAxis(
                            ap=ix[:rows, :1], axis=0),
                    )
                    for cc in range(2):
                        tp = tps.tile([128, 128], BF16, tag="tpb")
                        nc.tensor.transpose(
                            tp[:, :rows],
                            xg[:rows, cc * 128:(cc + 1) * 128],
                            idt[:rows, :rows])
                        nc.vector.tensor_copy(
                            xt_sb[:, cc * TB + r0: cc * TB + r0 + rows],
                            tp[:, :rows])

                # ================= phase P: minimal preamble ============
                with tc.tile_pool(name="p_tp", bufs=2, space="PSUM") as tps:
                    # gathers for t-block 0, then W_x + Zx block 0 so the
                    # recurrence can start; remaining gathers follow.
                    for i in range(4):
                        emit_gather(i, tps)

                    # --- h0/c0 transposed init ---
                    hc_sb = psb.tile([B, U], F32, tag="hc")
                    nc.sync.dma_start(hc_sb[:, :], h0[:, :])
                    cc_sb = psb.tile([B, U], F32, tag="hc2")
                    nc.sync.dma_start(cc_sb[:, :], c0[:, :])
                    for k in range(4):
                        tp = tps.tile([128, 128], F32, tag="tp")
                        nc.tensor.transpose(tp[:, :B],
                                            hc_sb[:B, k * 128:(k + 1) * 128],
                                            idtf[:B, :B])
                        nc.vector.tensor_copy(Hk[:, k, 0, :], tp[:, :B])
                        nc.vector.tensor_scalar_mul(H8k[:, k, 0, :],
                                                    tp[:, :B], SA)
                        tp2 = tps.tile([128, 128], F32, tag="tp")
                        nc.tensor.transpose(tp2[:, :B],
                                            cc_sb[:B, k * 128:(k + 1) * 128],
                                            idtf[:B, :B])
                        nc.vector.tensor_copy(cT[:, k * B:(k + 1) * B],
                                              tp2[:, :B])

                    # --- W_x load + Zx block 0 (rest interleaved in R) ---
                    for k in range(2):
                        nc.sync.dma_start(wx_sb[:, k * G4U:(k + 1) * G4U],
                                          wx_bf[k * 128:(k + 1) * 128, :])
                    nc.sync.dma_start(wxb_sb[:, :], wx_bf[E:E + 1, :])
                    for mi in range(16):
                        emit_zx_unit(0, mi)

                    for i in range(4, 16):
                        emit_gather(i, tps)

                # big weight loads (first needed at R step 3)
                for k in range(8):
                    nc.sync.dma_start(wfc_sb[:, k * VS:(k + 1) * VS],
                                      wfc_q[k * 128:(k + 1) * 128, :])
                nc.sync.dma_start(bfc_bc[:], bfc_bf.to_broadcast([128, VS]))

                # ================= phase R: recurrence =================
                with ExitStack() as rr:
                    rsb = rr.enter_context(tc.tile_pool(name="r_sbuf", bufs=3))
                    rps = rr.enter_context(
                        tc.tile_pool(name="r_psum", bufs=2, space="PSUM"))
                    rga = rr.enter_context(tc.tile_pool(name="r_gate", bufs=2))
                    hps = rr.enter_context(
                        tc.tile_pool(name="r_hfc_ps", bufs=1, space="PSUM"))
                    hsb = rr.enter_context(tc.tile_pool(name="r_hfc_sb", bufs=2))
                    HQ = {}

                    # psum-bank-aligned N slices of a 2000-wide half
                    QSL = [(0, 512), (512, 512), (1024, 512), (1536, 464)]

                    def hfc_phase(mi, ph):
                        """h-half of FC chunk mi in fp8 DoubleRow.

                        4 phases: (half, k-pair) in [(0,0),(0,1),(1,0),(1,1)];
                        k-pair j covers global k chunks 4+2j, 5+2j.
                        """
                        r0 = mi * 128
                        half, j = ph // 2, ph % 2
                        if ph % 2 == 0:
                            HQ[mi] = hps.tile([128, 2000], F32, tag="hq",
                                              name="hq")
                        fcp = HQ[mi]
                        lhs = H8r[:, 2 * j: 2 * j + 2,
                                  B + r0: B + r0 + 128]
                        for off, w in QSL:
                            nc.tensor.matmul(
                                fcp[:, off:off + w], lhs,
                                wfc_k[:, 4 + 2 * j: 6 + 2 * j,
                                      half * 2000 + off: half * 2000 + off + w],
                                start=(j == 0), stop=(j == 1),
                                perf_mode=PM.DoubleRow)
                        if j == 1:
                            hst = hsb.tile([128, 2000], BF16, tag="hst")
                            nc.vector.tensor_add(
                                hst[:], fcp[:],
                                bfc_bc[:, half * 2000:(half + 1) * 2000])
                            nc.sync.dma_start(
                                hfc_d[mi, :, half * 2000:(half + 1) * 2000],
                                hst[:])

                    c_prev = cT
                    for t in range(T):
                        zx = rsb.tile([128, 512], F32, tag="zx")
                        nc.sync.dma_start(
                            zx[:].rearrange("p (m b) -> p m b", m=16), zxt_d[t])
                        zps = rps.tile([128, 512], F32, tag="zt")
                        # gate layout host-permuted to [g, i, f, o]
                        zq = rga.tile([128, 512], F32, tag="zq")
                        gate = {}
                        for gi, fn in enumerate((AF.Tanh, AF.Sigmoid,
                                                 AF.Sigmoid, AF.Sigmoid)):
                            for m2 in range(4):
                                mi = gi * 4 + m2
                                for j in range(2):
                                    nc.tensor.matmul(
                                        zps[:, mi * B:(mi + 1) * B],
                                        wh8r[:, 2 * j: 2 * j + 2,
                                             mi * 128:(mi + 1) * 128],
                                        H8k[:, 2 * j: 2 * j + 2, t, :],
                                        start=(j == 0), stop=(j == 1),
                                        perf_mode=PM.DoubleRow)
                            sl = slice(gi * 128, (gi + 1) * 128)
                            # z = zx + (Wh.h)/(SWH*SA)
                            nc.vector.scalar_tensor_tensor(
                                zq[:, sl], zps[:, sl], 1.0 / (SWH * SA),
                                zx[:, sl], op0=ALU.mult, op1=ALU.add)
                            gt = rga.tile([128, 128], F32, tag=f"g{gi}",
                                          name=f"g{gi}")
                            nc.scalar.activation(gt[:], zq[:, sl], fn)
                            gate[gi] = gt
                            if gi == 1:      # i ready: i*tanh(g)
                                ig = rga.tile([128, 128], F32, tag="ig")
                                nc.vector.tensor_mul(ig[:], gate[1][:],
                                                     gate[0][:])
                            elif gi == 2:    # f ready: c = f*c + i*g
                                fc_ = rga.tile([128, 128], F32, tag="fc")
                                nc.vector.tensor_mul(fc_[:], gate[2][:],
                                                     c_prev[:])
                                c_new = rga.tile([128, 128], F32, tag="cn")
                                nc.vector.tensor_add(c_new[:], fc_[:], ig[:])
                                tc_ = rga.tile([128, 128], F32, tag="tc")
                                nc.scalar.activation(tc_[:], c_new[:], AF.Tanh)
                        nc.vector.tensor_mul(
                            Hk[:, :, t + 1, :],
                            gate[3][:].rearrange("p (k b) -> p k b", k=4),
                            tc_[:].rearrange("p (k b) -> p k b", k=4))
                        # fp8 x16 copy of h for the FC (one fused DVE op)
                        nc.vector.scalar_tensor_tensor(
                            H8k[:, :, t + 1, :],
                            gate[3][:].rearrange("p (k b) -> p k b", k=4),
                            SA,
                            tc_[:].rearrange("p (k b) -> p k b", k=4),
                            op0=ALU.mult, op1=ALU.mult)
                        c_prev = c_new
                        # interleaved background work for this step:
                        # Zx blocks 1-3 at steps 2..25 (2 m-chunks/step)
                        if 2 <= t <= 25:
                            g = t - 2
                            nb = 1 + g // 8
                            for sub in range(2):
                                emit_zx_unit(nb, (g % 8) * 2 + sub)
                        # h-part FC: chunk mi's 4 phases at steps 4mi+3..+6
                        if t >= 3:
                            g = t - 3
                            hfc_phase(g // 4, g % 4)

            # ===== phase D1: ep, scores/softmax/ctx (batched over t) =====
            with ExitStack() as d1:
                dsb = d1.enter_context(tc.tile_pool(name="d1_sbuf", bufs=2))
                dps = d1.enter_context(
                    tc.tile_pool(name="d1_psum", bufs=2, space="PSUM"))
                dst_ = d1.enter_context(tc.tile_pool(name="d1_stat", bufs=2))
                att_pool = d1.enter_context(tc.tile_pool(name="d1_att", bufs=1))
                enc_pool = d1.enter_context(tc.tile_pool(name="d1_enc", bufs=1))

                # --- ep.T = (enc @ Wa).T : [u-chunk, (b,l)] ---
                enct_sb = enc_pool.tile([128, 4 * BL], BF16)
                for k in range(4):
                    nc.sync.dma_start(enct_sb[:, k * BL:(k + 1) * BL],
                                      enct_bf[k * 128:(k + 1) * 128, :])
                wa_sb = enc_pool.tile([128, 4 * U], BF16)   # [k, k*512 + m]
                for k in range(4):
                    nc.sync.dma_start(wa_sb[:, k * U:(k + 1) * U],
                                      wa_bf[k * 128:(k + 1) * 128, :])
                for mu in range(4):
                    for nb in range(4):
                        eps_ = dps.tile([128, 512], F32, tag="ep")
                        for k in range(4):
                            nc.tensor.matmul(
                                eps_[:, :],
                                wa_sb[:, k * U + mu * 128:
                                      k * U + (mu + 1) * 128],
                                enct_sb[:, k * BL + nb * 512:
                                        k * BL + (nb + 1) * 512],
                                start=(k == 0), stop=(k == 3))
                        nc.vector.tensor_copy(
                            epT_sb[:, mu * BL + nb * 512:
                                   mu * BL + (nb + 1) * 512],
                            eps_[:])

                enc_sb = enc_pool.tile([128, 16 * U], BF16)  # 2 b per group
                for j in range(16):
                    nc.sync.dma_start(enc_sb[0:64, j * U:(j + 1) * U],
                                      enc_bf[2 * j, :, :])
                    nc.sync.dma_start(enc_sb[64:128, j * U:(j + 1) * U],
                                      enc_bf[2 * j + 1, :, :])
                attnT_sb = att_pool.tile([128, 16 * T], BF16)  # attn.T 2b/tile

                for j in range(16):          # pairs of b
                    scp = dps.tile([128, 64], F32, tag="sc")
                    for half in range(2):
                        b = 2 * j + half
                        po = 64 * half
                        for k in range(4):
                            nc.tensor.matmul(
                                scp[po:po + T, :],
                                Hk[:, k, 1:64, b],
                                epT_sb[:, k * BL + b * L:
                                       k * BL + (b + 1) * L],
                                start=(k == 0), stop=(k == 3))
                    att_f = dsb.tile([128, 64], F32, tag="af")
                    attb = dsb.tile([128, 64], BF16, tag="ab")
                    for half in range(2):
                        po = 64 * half
                        nmx = dst_.tile([128, 1], F32, tag="nm")
                        nc.vector.tensor_reduce(nmx[po:po + T, :],
                                                scp[po:po + T, :],
                                                axis=AX.X, op=ALU.max,
                                                negate=True)
                        ssum = dst_.tile([128, 1], F32, tag="ss")
                        nc.scalar.activation(att_f[po:po + T, :],
                                             scp[po:po + T, :],
                                             AF.Exp, bias=nmx[po:po + T, :],
                                             accum_out=ssum[po:po + T, :])
                        rcp = dst_.tile([128, 1], F32, tag="rc")
                        nc.vector.reciprocal(rcp[po:po + T, :],
                                             ssum[po:po + T, :])
                        nc.vector.tensor_scalar_mul(attb[po:po + T, :],
                                                    att_f[po:po + T, :],
                                                    rcp[po:po + T, :])
                    for half in range(2):
                        po = 64 * half
                        tpp = dps.tile([128, T], BF16, tag="tpa")
                        nc.tensor.transpose(tpp[po:po + L, :],
                                            attb[po:po + T, :L],
                                            idt[po:po + T, po:po + T])
                        nc.vector.tensor_copy(
                            attnT_sb[po:po + L, j * T:(j + 1) * T],
                            tpp[po:po + L, :])

                # ctx.T per b -> G_ctx (fp8, x16)
                for j in range(16):
                    for half in range(2):
                        b = 2 * j + half
                        po = 64 * half
                        for mu in range(4):
                            ctp = dps.tile([128, T], F32, tag="ctx")
                            nc.tensor.matmul(
                                ctp[:, :],
                                enc_sb[po:po + L,
                                       j * U + mu * 128: j * U + (mu + 1) * 128],
                                attnT_sb[po:po + L, j * T:(j + 1) * T],
                                start=True, stop=True)
                            nc.vector.tensor_scalar_mul(Gck[:, mu, :, b],
                                                        ctp[:, :], SA)

        # ===== phase D2: FC + stats + pipelined AllReduce/finalize =====
        with ExitStack() as d2:
            fps = d2.enter_context(
                tc.tile_pool(name="d2_psum", bufs=2, space="PSUM"))
            lgp = d2.enter_context(tc.tile_pool(name="d2_lg", bufs=10))
            hpp = d2.enter_context(tc.tile_pool(name="d2_hp", bufs=2))
            scr = d2.enter_context(tc.tile_pool(name="d2_scr", bufs=2))
            sst = d2.enter_context(tc.tile_pool(name="d2_st", bufs=2))
            fin = d2.enter_context(tc.tile_pool(name="d2_fin", bufs=2))
            HALF = VS // 2            # 2000
            QSL = [(0, 512), (512, 512), (1024, 512), (1536, 464)]
            lg_tiles = {}

            def ar_batch(bi):
                """AllReduce batch bi's sumexp; nlz = -(1 + ln S)."""
                ca, cb = 8 * bi, 8 * bi + 8
                nc.sync.dma_start(ccs_in[bi][:], lsum_sb[:, ca:cb])
                nc.gpsimd.collective_compute(
                    "AllReduce", ALU.add,
                    replica_groups=[list(range(n_cores))],
                    ins=[ccs_in[bi][:].opt()], outs=[ccs_out[bi][:].opt()])
                nc.gpsimd.dma_start(sg_sb[:, ca:cb], ccs_out[bi][:])
                nc.scalar.activation(logz_sb[:, ca:cb], sg_sb[:, ca:cb],
                                     AF.Ln)
                nc.vector.tensor_scalar(nlz_sb[:, ca:cb], logz_sb[:, ca:cb],
                                        -1.0, -1.0, op0=ALU.mult, op1=ALU.add)

            def fin_chunk(mi):
                """out = logits - (1 + lnS); alternate Vector/Scalar."""
                r0 = mi * 128
                rows = min(128, TB - r0)
                ob = fin.tile([128, VS], BF16, tag="ob")
                if mi % 2 == 0:
                    nc.vector.tensor_scalar(
                        ob[:rows, :], lg_tiles[mi][:rows, :],
                        nlz_sb[:rows, mi:mi + 1], None, op0=ALU.add)
                else:
                    nc.scalar.activation(
                        ob[:rows, :], lg_tiles[mi][:rows, :],
                        AF.Identity, bias=nlz_sb[:rows, mi:mi + 1])
                t0 = mi * 4
                for tl in range(rows // B):
                    nc.gpsimd.dma_start(out[:, t0 + tl, :],
                                        ob[tl * B:(tl + 1) * B, :])

            for mi in range(16):
                r0 = mi * 128
                rows = min(128, TB - r0)
                ac = [None, None]
                if mi < 15:
                    hpart = hpp.tile([128, VS], BF16, tag="hp")
                    nc.sync.dma_start(hpart[:], hfc_d[mi])
                lg = lgp.tile([128, VS], BF16, tag="lg")
                lg_tiles[mi] = lg
                for half in range(2):
                    fcp = fps.tile([128, HALF], F32, tag="fc")
                    # ctx half of FC: fp8 DoubleRow over k-pairs (0,1),(2,3)
                    jmax = 2 if mi < 15 else 4
                    for j in range(jmax):
                        lhs = Gcr[:, 2 * j: 2 * j + 2, r0:r0 + rows] \
                            if j < 2 else \
                            H8r[:, 2 * (j - 2): 2 * (j - 2) + 2,
                                B + r0: B + r0 + rows]
                        for off, w in QSL:
                            nc.tensor.matmul(
                                fcp[:rows, off:off + w],
                                lhs,
                                wfc_k[:, 2 * j: 2 * j + 2,
                                      half * HALF + off: half * HALF + off + w],
                                start=(j == 0), stop=(j == jmax - 1),
                                perf_mode=PM.DoubleRow)
                    if mi < 15:
                        # h-part (incl. b_fc) staged in bf16 (xSP) during R
                        nc.vector.tensor_add(
                            fcp[:rows, :], fcp[:rows, :],
                            hpart[:rows, half * HALF:(half + 1) * HALF])
                    else:
                        nc.vector.tensor_add(
                            fcp[:rows, :], fcp[:rows, :],
                            bfc_bc[:rows, half * HALF:(half + 1) * HALF])
                    nc.scalar.activation(
                        lg[:rows, half * HALF:(half + 1) * HALF],
                        fcp[:rows, :], AF.Tanh, scale=1.0 / SP)
                    # tanh bounds logits to [-1,1]: fixed shift exp(x-1)
                    sc_ = scr.tile([128, HALF], BF16, tag="sc")
                    acx = sst.tile([128, 1], F32, tag="ac")
                    nc.scalar.activation(sc_[:rows, :],
                                         lg[:rows, half * HALF:(half + 1) * HALF],
                                         AF.Exp, bias=negone[:rows, :],
                                         accum_out=acx[:rows, :])
                    ac[half] = acx
                nc.vector.tensor_add(lsum_sb[:rows, mi:mi + 1],
                                     ac[0][:rows, :], ac[1][:rows, :])
                if mi == 7:
                    ar_batch(0)
                if mi >= 10:
                    fin_chunk(mi - 10)      # batch-0 chunks 0..5
            for mi in range(6, 8):
                fin_chunk(mi)
            ar_batch(1)
            for mi in range(8, 16):
                fin_chunk(mi)

    nc.compile()
    return nc


def _bf(x):
    return np.ascontiguousarray(
        np.asarray(x, np.float32).astype(ml_dtypes.bfloat16))


def _q8(x, scale):
    y = np.asarray(x, np.float32) * scale
    y = np.clip(y, -240.0, 240.0)
    return np.ascontiguousarray(y.astype(ml_dtypes.float8_e4m3))


def prep_inputs(target, encoder_outputs, enc_h0, enc_c0, emb, W_x, W_h,
                b_lstm, Wa, W_fc, b_fc, n_cores=NC):
    """Host-side layout prep + per-core sharding."""
    tgt = np.asarray(target).astype(np.int32)
    tidx = np.ascontiguousarray(tgt[:, :T].T.reshape(TB, 1))  # t-major rows
    enc = np.asarray(encoder_outputs, np.float32)
    wx_ext = np.concatenate([np.asarray(W_x, np.float32),
                             np.asarray(b_lstm, np.float32)[None, :]], axis=0)
    # permute gate columns [i,f,g,o] -> [g,i,f,o]
    gperm = np.r_[2 * U:3 * U, 0:U, U:2 * U, 3 * U:4 * U]
    wx_ext = wx_ext[:, gperm]
    W_h = np.asarray(W_h, np.float32)[:, gperm]
    common = {
        "tidx": tidx,
        "emb_bf": _bf(emb),
        "wx_bf": _bf(wx_ext),
        "wh_q": _q8(W_h, SWH),
        "enc_bf": _bf(enc),
        "enct_bf": _bf(enc.transpose(2, 0, 1).reshape(U, BL)),
        "wa_bf": _bf(Wa),
        "h0": np.ascontiguousarray(np.asarray(enc_h0, np.float32)),
        "c0": np.ascontiguousarray(np.asarray(enc_c0, np.float32)),
    }
    wfc = np.asarray(W_fc, np.float32)
    bfc = np.asarray(b_fc, np.float32)
    in_maps = []
    for c in range(n_cores):
        m = dict(common)
        m["wfc_q"] = _q8(wfc[:, c * VS:(c + 1) * VS], SW)
        m["bfc_bf"] = _bf(bfc[c * VS:(c + 1) * VS].reshape(1, VS) * SP)
        in_maps.append(m)
    return in_maps


def kernel(**inputs):
    if "nc" not in _CACHE:
        _CACHE["nc"] = build(NC)
    nc = _CACHE["nc"]
    in_maps = prep_inputs(**inputs, n_cores=NC)
    res = run_bass_kernel_spmd(nc, in_maps, list(range(NC)))
    shards = [np.asarray(res.results[c]["out"]).astype(np.float32)
              for c in range(NC)]
    return np.concatenate(shards, axis=-1)
